# revision 25
# baseline (speedup 1.0000x reference)
"""Kent-distribution pairwise KLD loss kernel for Trainium2 (8 NeuronCores).

The [N, M] pairwise KLD matrix factors exactly as a rank-11 product
U @ V^T (see kernel_baseline.py for the derivation).  This version:

 - computes the 11 features per row in f32, casts to fp16, and runs the
   skinny matmuls in fp16 (PE fp32 runs dual-pass LOW_HIGH at ~4x the
   cost; fp16 features give ~7e-4 absmax-rel, gate is 2e-2)
 - pads features to 32 and uses 4-way PE row tiling (K=11 <= 32):
   4 batched transposes put 4 target groups at partition blocks
   {0,32,64,96}; the 8 main matmuls run 4-at-a-time in the array
 - stores V^T m-sorted in SBUF so the matmul moving operand is
   contiguous AND the PSUM->SBUF out-copies have a packed last dim
   (strided-singles dest measured ~5ns/elem vs ~1.3ns packed)
 - writes the output as fp16 (upcast on host), halving the output DMA
 - orders ACT ops in function phases (Sin..., then Ln, then copies):
   every Sin<->Ln alternation reloads a 1283ns activation table
 - splits the elementwise preamble across DVE and GpSimd with algebraic
   reductions: with D = k^2-4b^2, r = 1/D:
     l1 = 1 - k*r,  l2 = (k-1)*r - 1.5*(4b^2)*r^2,
     A  = 0.5*ln(D+eps) - k^2*r        (ln2pi cancels against c_b,
                                        |gamma_1|^2 == 1 exactly)

Feature vectors (rank 11), row n pred / row m target:
  U = [A, 1, l1*g1, l2+dE*g1^2, dE*(g1 pairs)]        dE = l1-l2
  V = [1, c_b', -kb*gb1, bb*(gb3^2-gb2^2), 2bb*(gb3 pairs - gb2 pairs)]
  c_b' = kb - 0.5*ln(Db+eps)
"""

import sys

import numpy as np

sys.path.insert(0, "/opt/trn_rl_repo")

import concourse.bass as bass  # noqa: E402,F401
import concourse.mybir as mybir  # noqa: E402
import concourse.tile as tile  # noqa: E402
from concourse import bacc  # noqa: E402
from concourse.masks import make_identity  # noqa: E402

F32 = mybir.dt.float32
F16 = mybir.dt.float16
AF = mybir.ActivationFunctionType
ALU = mybir.AluOpType

N = 2048
M = 2048
NCORES = 8
NS = N // NCORES  # 256 pred rows per core
GP = 2  # pred groups: row = 2p + j
GT = 16  # target groups: row = 16p + j
G = GP + GT  # 18 param columns
FP = 32  # padded feature stride (11 real features)

PI = float(np.pi)
EPS = 1e-6


def _body(tc, pred, targ, out):
    nc = tc.nc
    with (
        tc.tile_pool(name="main", bufs=1) as pool,
        tc.tile_pool(name="tp_psum", bufs=1, space="PSUM") as tpp,
        tc.tile_pool(name="mm_psum", bufs=6, space="PSUM") as opp,
    ):
        def t(shape, tag, dtype=F32):
            return pool.tile([128, *shape], dtype, name=tag, tag=tag)

        # ---------- constants / table warmup (param-independent) ----------
        half_pi = t([1], "half_pi")
        nc.vector.memset(half_pi, PI / 2)
        eps_c = t([1], "eps_c")
        nc.vector.memset(eps_c, EPS)
        # load the Sin table during the input-DMA wait (Ln loads once,
        # right before LNOUT -- keep ACT functions phase-ordered)
        sin_dummy = t([1], "sin_dummy")
        nc.scalar.activation(sin_dummy[:], half_pi[:], AF.Sin)

        identf = t([128], "identf")
        make_identity(nc, identf[:])
        ident = t([128], "ident", F16)
        nc.vector.tensor_copy(ident[:], identf[:])

        # constant tiles so the GpSimd chain needs only plain tensor_tensor
        ones2 = t([GP], "ones2")
        nc.gpsimd.memset(ones2, 1.0)
        cm15 = t([GP], "cm15")
        nc.gpsimd.memset(cm15, -1.5)

        # feature tiles (fp16), zero-padded to 32; feature 0/1 constants.
        # VF flat free dim = (b, a, f): feature f of target group g = 4a + b,
        # so transpose b reads the contiguous slice [:, 128b:128(b+1)].
        VF_flat = t([4 * 4 * FP], "VF", F16)  # [128, 512] target features
        VF = VF_flat.rearrange("p (b a f) -> p b a f", b=4, a=4)
        nc.gpsimd.memset(VF_flat[:], 0.0)
        nc.gpsimd.memset(VF[:, :, :, 0], 1.0)
        UF = t([GP, FP], "UF", F16)  # [128, 2, 32] pred features
        nc.gpsimd.memset(UF[:], 0.0)
        nc.gpsimd.memset(UF[:, :, 1], 1.0)

        def tg(x):
            # target-group-indexed [128, 16] -> (b, a) iteration order
            return x.rearrange("p (a b) -> p b a", b=4)

        def tgc(x):
            # [128, c, 16] -> (b, a, c) iteration order
            return x.rearrange("p c (a b) -> p b a c", b=4)

        # ---------- load params ----------
        # pred partition p holds rows 2p,2p+1; targ partition p rows 16p..16p+15
        params = t([G * 5], "params")
        nc.sync.dma_start(
            out=params[:, GP * 5 : G * 5],
            in_=targ.rearrange("(p j) c -> p (j c)", p=128),
        )
        nc.scalar.dma_start(
            out=params[:, 0 : GP * 5],
            in_=pred.rearrange("(p j) c -> p (j c)", p=128),
        )

        P5 = params.rearrange("p (g c) -> p c g", c=5)  # [128, 5, 18]
        angles = P5[:, 0:3, :]
        kap = P5[:, 3, :]
        bet = P5[:, 4, :]
        kap_t, bet_t = kap[:, GP:G], bet[:, GP:G]
        kap_p = kap[:, 0:GP]

        # ---------- trig (Sin domain is [-pi, pi]) ----------
        absv = t([3, G], "absv")
        nc.vector.scalar_tensor_tensor(absv[:], angles, -1.0, angles, ALU.mult, ALU.max)
        trig = t([3, 2, G], "trig")  # [angle, (cos, sin), group]
        nc.scalar.activation(trig[:, :, 1, :], angles, AF.Sin)
        nc.scalar.activation(trig[:, :, 0, :], absv[:], AF.Sin, bias=half_pi, scale=-1.0)
        ce, se = trig[:, 0, 0, :], trig[:, 0, 1, :]
        ca, sa = trig[:, 1, 0, :], trig[:, 1, 1, :]
        cp, sp = trig[:, 2, 0, :], trig[:, 2, 1, :]
        cs_e = trig[:, 0, :, :]  # (ce, se)
        cs_p = trig[:, 2, :, :]  # (cp, sp)

        # ---------- GpSimd: kappa/beta slabs ----------
        b2 = t([G], "b2")
        nc.gpsimd.tensor_add(b2, bet, bet)
        s4 = t([G], "s4")
        nc.gpsimd.tensor_mul(s4, b2, b2)
        x2 = t([G], "x2")
        nc.gpsimd.tensor_mul(x2, kap, kap)
        D = t([G], "D")
        nc.gpsimd.tensor_sub(D, x2, s4)
        b2_t = b2[:, GP:G]

        # ACT: the two f32 copies feed the DVE product chain; then the one
        # Sin->Ln function switch of the kernel
        g1 = t([3, G], "g1")
        mm4 = t([4, G], "mm4")
        nc.scalar.copy(mm4[:, 2:4, :], cs_p)
        nc.scalar.copy(g1[:, 0, :], ca)
        LNOUT = t([G], "LNOUT")
        nc.scalar.activation(LNOUT[:], D[:], AF.Ln, bias=eps_c)

        # reciprocal is DVE-only
        r = t([GP], "r")
        nc.vector.reciprocal(r, D[:, 0:GP])

        # ---------- DVE: trig products ----------
        # g1 = [ca, sa*ce, sa*se] for all 18 groups
        nc.vector.tensor_mul(
            g1[:, 1:3, :], sa.unsqueeze(1).broadcast_to([128, 2, G]), cs_e
        )
        # mm4 = [m2=ca*cp, m4=ca*sp, cp, sp]
        nc.vector.tensor_mul(
            mm4[:, 0:2, :], ca.unsqueeze(1).broadcast_to([128, 2, G]), cs_p
        )
        # s8[i, j] = mm4_i * (ce if j==0 else se)
        s8 = t([4, 2, G], "s8")
        nc.vector.tensor_mul(
            s8[:],
            mm4[:].unsqueeze(2).broadcast_to([128, 4, 2, G]),
            cs_e.unsqueeze(1).broadcast_to([128, 4, 2, G]),
        )
        sa_t = trig[:, 1, 1, GP:G]

        # g2 = [-cp sa, m2 ce - sp se, m2 se + sp ce]
        # g3 = [ sp sa, -(m4 ce + cp se), cp ce - m4 se]      (targets only)
        g23 = t([2, 3, GT], "g23")
        nc.vector.scalar_tensor_tensor(
            g23[:, 0, 0, :], cp[:, GP:G], -1.0, sa_t, ALU.mult, ALU.mult
        )
        nc.vector.scalar_tensor_tensor(
            g23[:, 1, 1, :], s8[:, 1, 0, GP:G], -1.0, s8[:, 2, 1, GP:G],
            ALU.mult, ALU.subtract,
        )
        # the four plain-TT g23 entries run on GpSimd (it idles here; the
        # lambda chain below waits on r anyway)
        nc.gpsimd.tensor_mul(g23[:, 1, 0, :], sp[:, GP:G], sa_t)
        nc.gpsimd.tensor_sub(g23[:, 0, 1, :], s8[:, 0, 0, GP:G], s8[:, 3, 1, GP:G])
        nc.gpsimd.tensor_add(g23[:, 0, 2, :], s8[:, 0, 1, GP:G], s8[:, 3, 0, GP:G])
        nc.gpsimd.tensor_sub(g23[:, 1, 2, :], s8[:, 2, 0, GP:G], s8[:, 1, 1, GP:G])

        # ---------- GpSimd: lambda chain ([128, 2]) ----------
        u = t([GP], "u")
        nc.gpsimd.tensor_mul(u, kap_p, r)
        v2 = t([GP], "v2")
        nc.gpsimd.tensor_mul(v2, s4[:, 0:GP], r)
        h15 = t([GP], "h15")
        nc.gpsimd.tensor_mul(h15, v2, cm15)
        w = t([GP], "w")
        nc.gpsimd.tensor_add(w, h15, kap_p)
        l2t = t([GP], "l2t")
        nc.gpsimd.tensor_mul(l2t, w, r)
        l2 = t([GP], "l2")
        nc.gpsimd.tensor_sub(l2, l2t, r)
        l1 = t([GP], "l1")
        nc.gpsimd.tensor_sub(l1, ones2, u)
        dE = t([GP], "dE")
        nc.gpsimd.tensor_sub(dE, l1, l2)
        ku = t([GP], "ku")
        nc.gpsimd.tensor_mul(ku, kap_p, u)

        # ---------- DVE: pair products, dV, feature writes ----------
        pdc = t([2, 6, GT], "pdc")
        nc.vector.tensor_mul(pdc[:, :, 0:3, :], g23[:], g23[:])
        nc.vector.tensor_mul(
            pdc[:, :, 3:5, :],
            g23[:, :, 0:1, :].broadcast_to([128, 2, 2, GT]),
            g23[:, :, 1:3, :],
        )
        nc.vector.tensor_mul(pdc[:, :, 5, :], g23[:, :, 1, :], g23[:, :, 2, :])
        dV = t([6, GT], "dV")
        nc.vector.tensor_sub(dV[:], pdc[:, 1, :, :], pdc[:, 0, :, :])

        # target features 2..10 (fp16 writes, (b, a, feat) layout)
        nc.vector.tensor_mul(
            VF[:, :, :, 5:8],
            tg(bet_t).unsqueeze(3).broadcast_to([128, 4, 4, 3]),
            tgc(dV[:, 0:3, :]),
        )
        nc.vector.tensor_mul(
            VF[:, :, :, 8:11],
            tg(b2_t).unsqueeze(3).broadcast_to([128, 4, 4, 3]),
            tgc(dV[:, 3:6, :]),
        )
        negk = t([GT], "negk")
        nc.vector.tensor_scalar_mul(negk, kap_t, -1.0)
        nc.vector.tensor_mul(
            VF[:, :, :, 2:5],
            tg(negk[:]).unsqueeze(3).broadcast_to([128, 4, 4, 3]),
            tgc(g1[:, :, GP:G]),
        )
        # V feature 1 and the whole pred-feature tail are demoted so the
        # scheduler cannot emit them ahead of the VF-critical DVE chain
        # (observed head-of-line stalls on the LNOUT/ku semaphore waits)
        lowprio = tc.high_priority(offset=-100000)
        lowprio.__enter__()
        nc.vector.scalar_tensor_tensor(
            VF[:, :, :, 1], tg(LNOUT[:, GP:G]), -0.5, tg(kap_t), ALU.mult, ALU.add
        )

        # ---------- DVE: pred features ----------
        g1p = g1[:, :, 0:GP]
        q6 = t([6, GP], "q6")
        nc.vector.tensor_mul(q6[:, 0:3, :], g1p, g1p)
        nc.vector.tensor_mul(
            q6[:, 3:5, :], g1[:, 0:1, 0:GP].broadcast_to([128, 2, GP]), g1[:, 1:3, 0:GP]
        )
        nc.vector.tensor_mul(q6[:, 5, :], g1[:, 1, 0:GP], g1[:, 2, 0:GP])
        tq = t([3, GP], "tq")
        nc.vector.tensor_mul(
            tq[:], q6[:, 0:3, :], dE.unsqueeze(1).broadcast_to([128, 3, GP])
        )
        nc.vector.tensor_add(
            UF[:, :, 5:8],
            tq[:].rearrange("p c g -> p g c"),
            l2.unsqueeze(2).broadcast_to([128, GP, 3]),
        )
        nc.vector.tensor_mul(
            UF[:, :, 8:11],
            q6[:, 3:6, :].rearrange("p c g -> p g c"),
            dE.unsqueeze(2).broadcast_to([128, GP, 3]),
        )
        nc.vector.tensor_mul(
            UF[:, :, 2:5],
            g1p.rearrange("p c g -> p g c"),
            l1.unsqueeze(2).broadcast_to([128, GP, 3]),
        )
        # U feature 0: A = 0.5 ln(D+eps) - k^2 r
        nc.vector.scalar_tensor_tensor(
            UF[:, :, 0], LNOUT[:, 0:GP], 0.5, ku, ALU.mult, ALU.subtract
        )
        # replicate UF to all 4 partition blocks for row-tiled LDWEIGHTS
        ufr = t([GP * 4 * FP], "ufr", F16)  # flat (ti, b, f)
        nc.vector.tensor_copy(
            ufr.rearrange("p (t b f) -> p t b f", t=GP, b=4),
            UF[:].unsqueeze(2).broadcast_to([128, GP, 4, FP]),
        )

        lowprio.__exit__(None, None, None)

        # ---------- transposes: feature-major -> row-tiled layout ----------
        # transpose b reads VF cols (b, a, f); out partition 32a+f = feature f
        # of group 4a+b, free col = target p
        vtp = tpp.tile([128, 512], F16, name="vtp", tag="vtp")
        for tt in range(4):
            nc.tensor.transpose(
                vtp[:, 128 * tt : 128 * (tt + 1)],
                VF_flat[:, 128 * tt : 128 * (tt + 1)],
                ident[:],
            )
        utp = tpp.tile([128, 256], F16, name="utp", tag="utp")
        for ti in range(GP):
            nc.tensor.transpose(
                utp[:, 128 * ti : 128 * (ti + 1)],
                ufr[:, 128 * ti : 128 * (ti + 1)],
                ident[:],
            )

        # VT_sb stored m-sorted: col (p, cb) = V^T of target row 16p + 4a + cb
        # at partition block a.  Copy src iterates vtp strided; dest last dim
        # is packed pairs so the copy keeps the fast path, and the matmul
        # moving operand below is fully contiguous.
        VT_sb = t([512], "VT_sb", F16)
        VT_dst = VT_sb.rearrange("k (p cb) -> k p cb", cb=4)  # [128,128,4]
        for h in range(2):
            src = vtp[:, 256 * h : 256 * (h + 1)].rearrange(
                "k (cb p) -> k p cb", p=128
            )
            eng = [nc.scalar, nc.vector][h]
            copy_fn = eng.copy if eng is nc.scalar else eng.tensor_copy
            copy_fn(VT_dst[:, :, 2 * h : 2 * h + 2], src)
        # UT_sb[32b+f, 128ti+p] = U feature f of pred row 2p + ti  (all b)
        UT_sb = t([256], "UT_sb", F16)
        nc.vector.tensor_copy(UT_sb[:], utp[:])

        # ---------- main matmuls: 4-way row-tiled, K=11 ----------
        outv = out.rearrange("(p t) m -> p t m", t=GP)  # row = 2p + ti
        copy_engines = [nc.vector, nc.scalar]
        for ti in range(GP):
            out_sb = t([2048], f"out_sb{ti}", F16)
            out_sbv = out_sb.rearrange("p (q j) -> p q j", j=GT)  # [128,128,16]
            for b in range(4):
                ops = opp.tile([128, 512], F32, name="ops", tag="ops")
                nc.tensor.matmul(
                    ops[:],
                    UT_sb[32 * b : 32 * b + 11, 128 * ti : 128 * (ti + 1)],
                    VT_sb[32 * b : 32 * b + 11, :],
                    start=True,
                    stop=True,
                    tile_position=(32 * b, 0),
                )
                # chunk b column (p, cb) -> m_local = 16p + 4b + cb
                eng = copy_engines[(4 * ti + b) % 2]
                copy_fn = eng.copy if eng is nc.scalar else eng.tensor_copy
                copy_fn(
                    out_sbv[:, :, 4 * b : 4 * b + 4],
                    ops.rearrange("p (q j) -> p q j", j=4),
                )
            nc.sync.dma_start(out=outv[:, ti, :], in_=out_sb[:])


def build():
    nc = bacc.Bacc()
    pred = nc.dram_tensor("pred", [NS, 5], F32, kind="ExternalInput")
    targ = nc.dram_tensor("targ", [M, 5], F32, kind="ExternalInput")
    out = nc.dram_tensor("out", [NS, M], F16, kind="ExternalOutput")
    with tile.TileContext(nc) as tc:
        _body(tc, pred[:], targ[:], out[:])
    nc.finalize()
    return nc


_NC_CACHE = None


def _get_nc():
    global _NC_CACHE
    if _NC_CACHE is None:
        _NC_CACHE = build()
    return _NC_CACHE


def kernel(kent_pred, kent_target, trace=False, tmpdir=None):
    from concourse.bass_utils import run_bass_kernel_spmd

    nc = _get_nc()
    kent_pred = np.ascontiguousarray(np.asarray(kent_pred, dtype=np.float32))
    kent_target = np.ascontiguousarray(np.asarray(kent_target, dtype=np.float32))
    in_maps = [
        {"pred": kent_pred[i * NS : (i + 1) * NS], "targ": kent_target}
        for i in range(NCORES)
    ]
    res = run_bass_kernel_spmd(
        nc, in_maps, core_ids=list(range(NCORES)), trace=trace, tmpdir=tmpdir
    )
    out = np.concatenate([r["out"] for r in res.results], axis=0).astype(np.float32)
    if trace:
        kernel.last_results = res
    return out


# revision 27
# speedup vs baseline: 1.0066x; 1.0066x over previous
"""Kent-distribution pairwise KLD loss kernel for Trainium2 (8 NeuronCores).

The [N, M] pairwise KLD matrix factors exactly as a rank-11 product
U @ V^T (see kernel_baseline.py for the derivation).  This version:

 - computes the 11 features per row in f32, casts to fp16, and runs the
   skinny matmuls in fp16 (PE fp32 runs dual-pass LOW_HIGH at ~4x the
   cost; fp16 features give ~7e-4 absmax-rel, gate is 2e-2)
 - pads features to 32 and uses 4-way PE row tiling (K=11 <= 32):
   4 batched transposes put 4 target groups at partition blocks
   {0,32,64,96}; the 8 main matmuls run 4-at-a-time in the array
 - stores V^T m-sorted in SBUF so the matmul moving operand is
   contiguous AND the PSUM->SBUF out-copies have a packed last dim
   (strided-singles dest measured ~5ns/elem vs ~1.3ns packed)
 - writes the output as fp16 (upcast on host), halving the output DMA
 - orders ACT ops in function phases (Sin..., then Ln, then copies):
   every Sin<->Ln alternation reloads a 1283ns activation table
 - splits the elementwise preamble across DVE and GpSimd with algebraic
   reductions: with D = k^2-4b^2, r = 1/D:
     l1 = 1 - k*r,  l2 = (k-1)*r - 1.5*(4b^2)*r^2,
     A  = 0.5*ln(D+eps) - k^2*r        (ln2pi cancels against c_b,
                                        |gamma_1|^2 == 1 exactly)

Feature vectors (rank 11), row n pred / row m target:
  U = [A, 1, l1*g1, l2+dE*g1^2, dE*(g1 pairs)]        dE = l1-l2
  V = [1, c_b', -kb*gb1, bb*(gb3^2-gb2^2), 2bb*(gb3 pairs - gb2 pairs)]
  c_b' = kb - 0.5*ln(Db+eps)
"""

import sys

import numpy as np

sys.path.insert(0, "/opt/trn_rl_repo")

import concourse.bass as bass  # noqa: E402,F401
import concourse.mybir as mybir  # noqa: E402
import concourse.tile as tile  # noqa: E402
from concourse import bacc  # noqa: E402
from concourse.masks import make_identity  # noqa: E402

F32 = mybir.dt.float32
F16 = mybir.dt.float16
AF = mybir.ActivationFunctionType
ALU = mybir.AluOpType

N = 2048
M = 2048
NCORES = 8
NS = N // NCORES  # 256 pred rows per core
GP = 2  # pred groups: row = 2p + j
GT = 16  # target groups: row = 16p + j
G = GP + GT  # 18 param columns
FP = 32  # padded feature stride (11 real features)

PI = float(np.pi)
EPS = 1e-6


def _body(tc, pred, targ, out):
    nc = tc.nc
    with (
        tc.tile_pool(name="main", bufs=1) as pool,
        tc.tile_pool(name="tp_psum", bufs=1, space="PSUM") as tpp,
        tc.tile_pool(name="mm_psum", bufs=6, space="PSUM") as opp,
    ):
        def t(shape, tag, dtype=F32):
            return pool.tile([128, *shape], dtype, name=tag, tag=tag)

        # ---------- constants / table warmup (param-independent) ----------
        half_pi = t([1], "half_pi")
        nc.vector.memset(half_pi, PI / 2)
        eps_c = t([1], "eps_c")
        nc.vector.memset(eps_c, EPS)
        # load the Sin table during the input-DMA wait (Ln loads once,
        # right before LNOUT -- keep ACT functions phase-ordered)
        sin_dummy = t([1], "sin_dummy")
        nc.scalar.activation(sin_dummy[:], half_pi[:], AF.Sin)

        identf = t([128], "identf")
        make_identity(nc, identf[:])
        ident = t([128], "ident", F16)
        nc.vector.tensor_copy(ident[:], identf[:])

        # constant tiles so the GpSimd chain needs only plain tensor_tensor
        ones2 = t([GP], "ones2")
        nc.gpsimd.memset(ones2, 1.0)
        cm15 = t([GP], "cm15")
        nc.gpsimd.memset(cm15, -1.5)
        cm05 = t([G], "cm05")
        nc.gpsimd.memset(cm05, -0.5)
        zer2 = t([GP], "zer2")
        nc.gpsimd.memset(zer2, 0.0)

        # feature tiles (fp16), zero-padded to 32; feature 0/1 constants.
        # VF flat free dim = (b, a, f): feature f of target group g = 4a + b,
        # so transpose b reads the contiguous slice [:, 128b:128(b+1)].
        VF_flat = t([4 * 4 * FP], "VF", F16)  # [128, 512] target features
        VF = VF_flat.rearrange("p (b a f) -> p b a f", b=4, a=4)
        nc.gpsimd.memset(VF_flat[:], 0.0)
        nc.gpsimd.memset(VF[:, :, :, 0], 1.0)
        UF = t([GP, FP], "UF", F16)  # [128, 2, 32] pred features
        nc.gpsimd.memset(UF[:], 0.0)
        nc.gpsimd.memset(UF[:, :, 1], 1.0)

        def tg(x):
            # target-group-indexed [128, 16] -> (b, a) iteration order
            return x.rearrange("p (a b) -> p b a", b=4)

        def tgc(x):
            # [128, c, 16] -> (b, a, c) iteration order
            return x.rearrange("p c (a b) -> p b a c", b=4)

        # ---------- load params ----------
        # pred partition p holds rows 2p,2p+1; targ partition p rows 16p..16p+15
        params = t([G * 5], "params")
        nc.sync.dma_start(
            out=params[:, GP * 5 : G * 5],
            in_=targ.rearrange("(p j) c -> p (j c)", p=128),
        )
        nc.scalar.dma_start(
            out=params[:, 0 : GP * 5],
            in_=pred.rearrange("(p j) c -> p (j c)", p=128),
        )

        P5 = params.rearrange("p (g c) -> p c g", c=5)  # [128, 5, 18]
        angles = P5[:, 0:3, :]
        kap = P5[:, 3, :]
        bet = P5[:, 4, :]
        kap_t, bet_t = kap[:, GP:G], bet[:, GP:G]
        kap_p = kap[:, 0:GP]

        # ---------- trig (Sin domain is [-pi, pi]) ----------
        absv = t([3, G], "absv")
        nc.vector.scalar_tensor_tensor(absv[:], angles, -1.0, angles, ALU.mult, ALU.max)
        trig = t([3, 2, G], "trig")  # [angle, (cos, sin), group]
        nc.scalar.activation(trig[:, :, 1, :], angles, AF.Sin)
        nc.scalar.activation(trig[:, :, 0, :], absv[:], AF.Sin, bias=half_pi, scale=-1.0)
        ce, se = trig[:, 0, 0, :], trig[:, 0, 1, :]
        ca, sa = trig[:, 1, 0, :], trig[:, 1, 1, :]
        cp, sp = trig[:, 2, 0, :], trig[:, 2, 1, :]
        cs_e = trig[:, 0, :, :]  # (ce, se)
        cs_p = trig[:, 2, :, :]  # (cp, sp)

        # ---------- GpSimd: kappa/beta slabs ----------
        b2 = t([G], "b2")
        nc.gpsimd.tensor_add(b2, bet, bet)
        s4 = t([G], "s4")
        nc.gpsimd.tensor_mul(s4, b2, b2)
        x2 = t([G], "x2")
        nc.gpsimd.tensor_mul(x2, kap, kap)
        D = t([G], "D")
        nc.gpsimd.tensor_sub(D, x2, s4)
        b2_t = b2[:, GP:G]

        # ACT: the two f32 copies feed the DVE product chain; then the one
        # Sin->Ln function switch of the kernel
        g1 = t([3, G], "g1")
        mm4 = t([4, G], "mm4")
        nc.scalar.copy(mm4[:, 2:4, :], cs_p)
        nc.scalar.copy(g1[:, 0, :], ca)
        LNOUT = t([G], "LNOUT")
        nc.scalar.activation(LNOUT[:], D[:], AF.Ln, bias=eps_c)

        # reciprocal is DVE-only
        r = t([GP], "r")
        nc.vector.reciprocal(r, D[:, 0:GP])

        # ---------- DVE: trig products ----------
        # g1 = [ca, sa*ce, sa*se] for all 18 groups
        nc.vector.tensor_mul(
            g1[:, 1:3, :], sa.unsqueeze(1).broadcast_to([128, 2, G]), cs_e
        )
        # mm4 = [m2=ca*cp, m4=ca*sp, cp, sp]
        nc.vector.tensor_mul(
            mm4[:, 0:2, :], ca.unsqueeze(1).broadcast_to([128, 2, G]), cs_p
        )
        # s8[i, j] = mm4_i * (ce if j==0 else se)
        s8 = t([4, 2, G], "s8")
        nc.vector.tensor_mul(
            s8[:],
            mm4[:].unsqueeze(2).broadcast_to([128, 4, 2, G]),
            cs_e.unsqueeze(1).broadcast_to([128, 4, 2, G]),
        )
        sa_t = trig[:, 1, 1, GP:G]

        # g2 = [-cp sa, m2 ce - sp se, m2 se + sp ce]
        # g3 = [ sp sa, -(m4 ce + cp se), cp ce - m4 se]      (targets only)
        g23 = t([2, 3, GT], "g23")
        nc.vector.scalar_tensor_tensor(
            g23[:, 0, 0, :], cp[:, GP:G], -1.0, sa_t, ALU.mult, ALU.mult
        )
        nc.vector.scalar_tensor_tensor(
            g23[:, 1, 1, :], s8[:, 1, 0, GP:G], -1.0, s8[:, 2, 1, GP:G],
            ALU.mult, ALU.subtract,
        )
        nc.vector.tensor_mul(g23[:, 1, 0, :], sp[:, GP:G], sa_t)
        nc.vector.tensor_sub(g23[:, 0, 1, :], s8[:, 0, 0, GP:G], s8[:, 3, 1, GP:G])
        nc.vector.tensor_add(g23[:, 0, 2, :], s8[:, 0, 1, GP:G], s8[:, 3, 0, GP:G])
        nc.vector.tensor_sub(g23[:, 1, 2, :], s8[:, 2, 0, GP:G], s8[:, 1, 1, GP:G])

        # ---------- GpSimd: lambda chain ([128, 2]) ----------
        u = t([GP], "u")
        nc.gpsimd.tensor_mul(u, kap_p, r)
        v2 = t([GP], "v2")
        nc.gpsimd.tensor_mul(v2, s4[:, 0:GP], r)
        h15 = t([GP], "h15")
        nc.gpsimd.tensor_mul(h15, v2, cm15)
        w = t([GP], "w")
        nc.gpsimd.tensor_add(w, h15, kap_p)
        l2t = t([GP], "l2t")
        nc.gpsimd.tensor_mul(l2t, w, r)
        l2 = t([GP], "l2")
        nc.gpsimd.tensor_sub(l2, l2t, r)
        l1 = t([GP], "l1")
        nc.gpsimd.tensor_sub(l1, ones2, u)
        dE = t([GP], "dE")
        nc.gpsimd.tensor_sub(dE, l1, l2)
        ku = t([GP], "ku")
        nc.gpsimd.tensor_mul(ku, kap_p, u)

        # ---------- DVE: pair products, dV, feature writes ----------
        pdc = t([2, 6, GT], "pdc")
        nc.vector.tensor_mul(pdc[:, :, 0:3, :], g23[:], g23[:])
        nc.vector.tensor_mul(
            pdc[:, :, 3:5, :],
            g23[:, :, 0:1, :].broadcast_to([128, 2, 2, GT]),
            g23[:, :, 1:3, :],
        )
        nc.vector.tensor_mul(pdc[:, :, 5, :], g23[:, :, 1, :], g23[:, :, 2, :])
        dV = t([6, GT], "dV")
        nc.vector.tensor_sub(dV[:], pdc[:, 1, :, :], pdc[:, 0, :, :])

        # target features 2..10 (fp16 writes, (b, a, feat) layout)
        nc.vector.tensor_mul(
            VF[:, :, :, 5:8],
            tg(bet_t).unsqueeze(3).broadcast_to([128, 4, 4, 3]),
            tgc(dV[:, 0:3, :]),
        )
        nc.vector.tensor_mul(
            VF[:, :, :, 8:11],
            tg(b2_t).unsqueeze(3).broadcast_to([128, 4, 4, 3]),
            tgc(dV[:, 3:6, :]),
        )
        negk = t([GT], "negk")
        nc.vector.tensor_scalar_mul(negk, kap_t, -1.0)
        nc.vector.tensor_mul(
            VF[:, :, :, 2:5],
            tg(negk[:]).unsqueeze(3).broadcast_to([128, 4, 4, 3]),
            tgc(g1[:, :, GP:G]),
        )
        # V feature 1 and the whole pred-feature tail are demoted so the
        # scheduler cannot emit them ahead of the VF-critical DVE chain
        # (observed head-of-line stalls on the LNOUT/ku semaphore waits)
        hl = t([G], "hl")
        nc.gpsimd.tensor_mul(hl, LNOUT[:], cm05[:])
        nc.gpsimd.tensor_add(VF[:, :, :, 1], tg(hl[:, GP:G]), tg(kap_t))
        lowprio = tc.high_priority(offset=-100000)
        lowprio.__enter__()

        # ---------- DVE: pred features ----------
        g1p = g1[:, :, 0:GP]
        q6 = t([6, GP], "q6")
        nc.vector.tensor_mul(q6[:, 0:3, :], g1p, g1p)
        nc.vector.tensor_mul(
            q6[:, 3:5, :], g1[:, 0:1, 0:GP].broadcast_to([128, 2, GP]), g1[:, 1:3, 0:GP]
        )
        nc.vector.tensor_mul(q6[:, 5, :], g1[:, 1, 0:GP], g1[:, 2, 0:GP])
        tq = t([3, GP], "tq")
        nc.vector.tensor_mul(
            tq[:], q6[:, 0:3, :], dE.unsqueeze(1).broadcast_to([128, 3, GP])
        )
        nc.vector.tensor_add(
            UF[:, :, 5:8],
            tq[:].rearrange("p c g -> p g c"),
            l2.unsqueeze(2).broadcast_to([128, GP, 3]),
        )
        nc.vector.tensor_mul(
            UF[:, :, 8:11],
            q6[:, 3:6, :].rearrange("p c g -> p g c"),
            dE.unsqueeze(2).broadcast_to([128, GP, 3]),
        )
        nc.vector.tensor_mul(
            UF[:, :, 2:5],
            g1p.rearrange("p c g -> p g c"),
            l1.unsqueeze(2).broadcast_to([128, GP, 3]),
        )
        # U feature 0: A = 0.5 ln(D+eps) - k^2 r = -(hl + ku) with hl = -ln/2
        hu = t([GP], "hu")
        nc.gpsimd.tensor_add(hu, hl[:, 0:GP], ku)
        nc.gpsimd.tensor_sub(UF[:, :, 0], zer2, hu)
        # replicate UF to all 4 partition blocks for row-tiled LDWEIGHTS
        ufr = t([GP * 4 * FP], "ufr", F16)  # flat (ti, b, f)
        nc.vector.tensor_copy(
            ufr.rearrange("p (t b f) -> p t b f", t=GP, b=4),
            UF[:].unsqueeze(2).broadcast_to([128, GP, 4, FP]),
        )

        lowprio.__exit__(None, None, None)

        # ---------- transposes: feature-major -> row-tiled layout ----------
        # transpose b reads VF cols (b, a, f); out partition 32a+f = feature f
        # of group 4a+b, free col = target p
        vtp = tpp.tile([128, 512], F16, name="vtp", tag="vtp")
        for tt in range(4):
            nc.tensor.transpose(
                vtp[:, 128 * tt : 128 * (tt + 1)],
                VF_flat[:, 128 * tt : 128 * (tt + 1)],
                ident[:],
            )
        utp = tpp.tile([128, 256], F16, name="utp", tag="utp")
        for ti in range(GP):
            nc.tensor.transpose(
                utp[:, 128 * ti : 128 * (ti + 1)],
                ufr[:, 128 * ti : 128 * (ti + 1)],
                ident[:],
            )

        # VT_sb stored m-sorted: col (p, cb) = V^T of target row 16p + 4a + cb
        # at partition block a.  Copy src iterates vtp strided; dest last dim
        # is packed pairs so the copy keeps the fast path, and the matmul
        # moving operand below is fully contiguous.
        VT_sb = t([512], "VT_sb", F16)
        VT_dst = VT_sb.rearrange("k (p cb) -> k p cb", cb=4)  # [128,128,4]
        for h in range(2):
            src = vtp[:, 256 * h : 256 * (h + 1)].rearrange(
                "k (cb p) -> k p cb", p=128
            )
            eng = [nc.scalar, nc.vector][h]
            copy_fn = eng.copy if eng is nc.scalar else eng.tensor_copy
            copy_fn(VT_dst[:, :, 2 * h : 2 * h + 2], src)
        # UT_sb[32b+f, 128ti+p] = U feature f of pred row 2p + ti  (all b)
        UT_sb = t([256], "UT_sb", F16)
        nc.vector.tensor_copy(UT_sb[:], utp[:])

        # ---------- main matmuls: 4-way row-tiled, K=11 ----------
        outv = out.rearrange("(p t) m -> p t m", t=GP)  # row = 2p + ti
        copy_engines = [nc.vector, nc.scalar]
        for ti in range(GP):
            out_sb = t([2048], f"out_sb{ti}", F16)
            out_sbv = out_sb.rearrange("p (q j) -> p q j", j=GT)  # [128,128,16]
            for b in range(4):
                ops = opp.tile([128, 512], F32, name="ops", tag="ops")
                nc.tensor.matmul(
                    ops[:],
                    UT_sb[32 * b : 32 * b + 11, 128 * ti : 128 * (ti + 1)],
                    VT_sb[32 * b : 32 * b + 11, :],
                    start=True,
                    stop=True,
                    tile_position=(32 * b, 0),
                )
                # chunk b column (p, cb) -> m_local = 16p + 4b + cb
                eng = copy_engines[(4 * ti + b) % 2]
                copy_fn = eng.copy if eng is nc.scalar else eng.tensor_copy
                copy_fn(
                    out_sbv[:, :, 4 * b : 4 * b + 4],
                    ops.rearrange("p (q j) -> p q j", j=4),
                )
            nc.sync.dma_start(out=outv[:, ti, :], in_=out_sb[:])


def build():
    nc = bacc.Bacc()
    pred = nc.dram_tensor("pred", [NS, 5], F32, kind="ExternalInput")
    targ = nc.dram_tensor("targ", [M, 5], F32, kind="ExternalInput")
    out = nc.dram_tensor("out", [NS, M], F16, kind="ExternalOutput")
    with tile.TileContext(nc) as tc:
        _body(tc, pred[:], targ[:], out[:])
    nc.finalize()
    return nc


_NC_CACHE = None


def _get_nc():
    global _NC_CACHE
    if _NC_CACHE is None:
        _NC_CACHE = build()
    return _NC_CACHE


def kernel(kent_pred, kent_target, trace=False, tmpdir=None):
    from concourse.bass_utils import run_bass_kernel_spmd

    nc = _get_nc()
    kent_pred = np.ascontiguousarray(np.asarray(kent_pred, dtype=np.float32))
    kent_target = np.ascontiguousarray(np.asarray(kent_target, dtype=np.float32))
    in_maps = [
        {"pred": kent_pred[i * NS : (i + 1) * NS], "targ": kent_target}
        for i in range(NCORES)
    ]
    res = run_bass_kernel_spmd(
        nc, in_maps, core_ids=list(range(NCORES)), trace=trace, tmpdir=tmpdir
    )
    out = np.concatenate([r["out"] for r in res.results], axis=0).astype(np.float32)
    if trace:
        kernel.last_results = res
    return out
